# revision 66
# baseline (speedup 1.0000x reference)
"""Bass/Trainium2 kernel for nn_BasicBlock_73933567033945 (CDConv / gnn_message_passing).

v3 "scatter-form" design. Graph is a fixed +-8 sequence window inside 4
chains of L=2048 nodes (asserted at runtime). 8192 nodes shard across 8
cores (1024 each), TS=112 output nodes per tile, 10 tiles, each backed
by a 128-row "slot" (core-halo rows 112t .. 112t+128).

Per tile, everything stays partition-aligned (the BIR verifier rejects
unaligned partition-offset operands; bases 32/64 hang the hardware):
 - the host marshals per-edge geometry features delta_aug into daT
   [(k,d) block-diag rows, slot-node cols] (17 window offsets + 1
   chain-boundary compensation slot folded via lrelu positive
   homogeneity, masks baked in).
 - 18 per-k pre-matmuls read daT with FREE-AXIS shifted views over a
   full-96-row contraction (ws rows outside each k's 8 lanes are zero),
   giving kernS_k[m, c] = kern[dest m-ksh] in source coordinates.
 - products tmS_k = kernS_k (bcast over w) * hX (h expanded over c) on
   DVE (plus 2/tile on gpsimd).
 - the per-k dest shift happens inside the PE transpose-accumulate:
   matmul(aggT += tmS_k^T @ S_k), S_k[m, j] = 1 iff j = m - ksh.
 - aggT ((w,c)-major) contracts with Wk (rows permuted to (w,c)) into
   conv; W_out and the identity add ride the same PSUM group.
All matmuls bf16 (fp32 streams 4x slower through the PE).
"""
import numpy as np
import ml_dtypes

B, L, C = 4, 2048, 128
N = B * L
W = 32
KC = 24
SEQ_L = 11
R = 12.0
WIN = 8
NEG_IN = 0.1
NEG_K = 0.2
NCORES = 8
NPC = N // NCORES          # 1024 nodes per core
TS = 112                   # output nodes per tile
NT = 10                    # tiles per core (9*112 + 16)
HR = (NT - 1) * TS + 128   # 1136 padded rows per core
K17 = 17
K18 = 18                   # 17 offsets + compensation slot
S_HALF = SEQ_L // 2
DTC = 6 * 144              # daT cols per tile (6 chunks x (128+16 pad))

BF = ml_dtypes.bfloat16

_PROG = {}


def _sidx(k):
    return int(np.clip(k - WIN, -S_HALF, S_HALF)) + S_HALF


def _ksh(k):
    return k if k < K17 else WIN


def _build_program():
    import concourse.tile as tile
    from concourse import mybir, bacc
    from contextlib import ExitStack

    f32 = mybir.dt.float32
    bf16 = mybir.dt.bfloat16
    AF = mybir.ActivationFunctionType
    OP = mybir.AluOpType

    nc = bacc.Bacc("TRN2", target_bir_lowering=False, debug=False)

    def din(name, shape, dt=f32):
        return nc.dram_tensor(name, shape, dt, kind="ExternalInput").ap()

    ws_in = din("ws_sb", [128, K18 * KC], bf16)
    idb_in = din("idb", [128, 128], bf16)
    wk_in = din("wk_p", [128, 6 * W], bf16)
    s_in = din("s_mats", [128, K18 * TS], bf16)
    a1_in = din("alph1", [128, 1])
    w_out_in = din("w_out", [W, C], bf16)
    hX_in = din("hX_slot", [128, NT * W * KC], bf16)
    daT_in = din("daT_slot", [128, NT * DTC], bf16)
    xc_in = din("xc_slot", [128, NT * C], bf16)
    y = nc.dram_tensor("y", [NPC, C], f32, kind="ExternalOutput").ap()

    with tile.TileContext(nc) as tc, ExitStack() as ctx:
        pers = ctx.enter_context(tc.tile_pool(name="pers", bufs=1))

        def load(ap_in, shape, tag, dt=f32):
            t = pers.tile(shape, dt, tag=tag)
            nc.sync.dma_start(t[:], ap_in)
            return t

        def load2(ap_in, shape, tag, dt=f32):
            t = pers.tile(shape, dt, tag=tag, name=tag)
            nc.scalar.dma_start(t[:], ap_in)
            return t

        # tile-0's inputs go FIRST on the sync queue; weights on the
        # scalar queue in parallel; remaining tiles stream on gpsimd
        hX_all = pers.tile([128, NT * W * KC], bf16, tag="hXa")
        daT_all = pers.tile([128, NT * DTC], bf16, tag="daT")
        xc_all = pers.tile([128, NT * C], bf16, tag="xc")
        HXC = W * KC
        nc.sync.dma_start(hX_all[:, 0:HXC], hX_in[:, 0:HXC])
        nc.sync.dma_start(daT_all[:, 0:DTC], daT_in[:, 0:DTC])
        nc.sync.dma_start(xc_all[:, 0:C], xc_in[:, 0:C])
        ws_sb = load(ws_in, [128, K18 * KC], "ws", bf16)
        a1_sb = load(a1_in, [128, 1], "a1")
        idb = load2(idb_in, [128, 128], "idb", bf16)
        wk_sb = load2(wk_in, [128, 6 * W], "wk", bf16)
        s_sb = load2(s_in, [128, K18 * TS], "smats", bf16)
        w_out_sb = load2(w_out_in, [W, C], "w_out", bf16)
        for t in range(1, NT):
            nc.gpsimd.dma_start(daT_all[:, DTC * t:DTC * (t + 1)],
                                daT_in[:, DTC * t:DTC * (t + 1)])
            nc.gpsimd.dma_start(hX_all[:, HXC * t:HXC * (t + 1)],
                                hX_in[:, HXC * t:HXC * (t + 1)])
            nc.gpsimd.dma_start(xc_all[:, C * t:C * (t + 1)],
                                xc_in[:, C * t:C * (t + 1)])

        wrk = ctx.enter_context(tc.tile_pool(name="wrk", bufs=2))
        tmp = ctx.enter_context(tc.tile_pool(name="tmp", bufs=6))
        ps_pre = ctx.enter_context(
            tc.tile_pool(name="ps_pre", bufs=2, space="PSUM"))
        ps_agg = ctx.enter_context(
            tc.tile_pool(name="ps_agg", bufs=2, space="PSUM"))
        ps_co = ctx.enter_context(
            tc.tile_pool(name="ps_co", bufs=2, space="PSUM"))

        AGGC = (0, 112, 224, 336, 512, 624)   # aggT chunk cols (bank-safe)

        def phase_a_kern(t):
            # h, hX, pre_k, kernS for slot t (emitted one tile ahead so the
            # PE's small pre-matmuls land before the previous tile's
            # transpose stream)
            pre_p = ps_pre.tile([128, 512], f32, tag="pre", name="pre_p")
            daTs = daT_all[:, DTC * t:DTC * (t + 1)]
            for k in range(K18):
                ksh = _ksh(k)
                c0 = 144 * (k // 3) + 16 - ksh
                # contract over the chunk's full 96 rows; ws_sb rows
                # outside this k's 8 lanes are zero, so bases stay 0
                nc.tensor.matmul(pre_p[:, KC * k:KC * (k + 1)],
                                 daTs[0:96, c0:c0 + 128],
                                 ws_sb[0:96, KC * k:KC * (k + 1)],
                                 start=True, stop=True,
                                 skip_group_check=True)
            kernS = wrk.tile([128, K18 * KC], bf16, tag="kernS",
                             name="kernS")
            nc.scalar.activation(kernS[:], pre_p[:, 0:K18 * KC], AF.Prelu,
                                 bias=0.0, scale=1.0, alpha=a1_sb[:, 0:1])
            return hX_all[:, HXC * t:HXC * (t + 1)], kernS

        cur = phase_a_kern(0)
        for t in range(NT):
            hX, kernS = cur
            if t + 1 < NT:
                cur = phase_a_kern(t + 1)
            ocnt = min(TS, NPC - TS * t)   # last tile has 16 real outputs

            # ---------------- products + transpose-accumulate -------------
            # comp slot k=17 shares S_8: add kernels, one fewer product
            kernC = wrk.tile([128, KC], bf16, tag="kernC")
            nc.vector.tensor_add(kernC[:], kernS[:, KC * 8:KC * 9],
                                 kernS[:, KC * 17:KC * 18])
            aggT_p = ps_agg.tile([128, 1024], f32, tag="aggT")

            # products in quad-k batches (amortize DVE init); k=8 single
            # (uses the folded kernC)
            QUADS = [(0, 4), (4, 8), (8, 9), (9, 13), (13, 17)]
            for (k0, k1) in QUADS:
                nk = k1 - k0
                tm4 = tmp.tile([128, 4 * W * KC], bf16, tag="tm",
                               name="tm")
                tm = tm4[:, 0:nk * W * KC]
                ksrc = kernC[:, 0:KC].rearrange("p (q c) -> p q c", q=1) \
                    if k0 == 8 else \
                    kernS[:, KC * k0:KC * k1].rearrange(
                        "p (q c) -> p q c", c=KC)
                nc.vector.tensor_mul(
                    tm.rearrange("p (q w c) -> p q w c", q=nk, c=KC),
                    hX.rearrange("p (w c) -> p w c", c=KC)
                        .unsqueeze(1).broadcast_to([128, nk, W, KC]),
                    ksrc.unsqueeze(2).broadcast_to([128, nk, W, KC]))
                for q in range(nk):
                    k = k0 + q
                    for b in range(6):
                        nc.tensor.matmul(
                            aggT_p[:, AGGC[b]:AGGC[b] + ocnt],
                            tm4[:, 768 * q + 128 * b:768 * q + 128 * (b + 1)],
                            s_sb[:, TS * k:TS * k + ocnt],
                            start=(k == 0 and b in (0, 4)),
                            stop=(k == K17 - 1 and b in (3, 5)),
                            skip_group_check=True)
            aggTs = wrk.tile([128, 736], bf16, tag="aggTs")
            nc.scalar.copy(aggTs[:, 0:448], aggT_p[:, 0:448])
            nc.scalar.copy(aggTs[:, 512:736], aggT_p[:, 512:736])

            # ---------------- conv = lrelu(Wk @ aggT); out ----------------
            co_p = ps_co.tile([128, 512], f32, tag="co")
            for b in range(6):
                nc.tensor.matmul(co_p[0:W, 0:ocnt],
                                 wk_sb[:, W * b:W * (b + 1)],
                                 aggTs[:, AGGC[b]:AGGC[b] + ocnt],
                                 start=(b == 0), stop=(b == 5),
                                 skip_group_check=True)
            convL = tmp.tile([W, TS], bf16, tag="convL")
            nc.scalar.activation(convL[0:W, 0:ocnt], co_p[0:W, 0:ocnt],
                                 AF.Prelu, bias=0.0, scale=1.0,
                                 alpha=a1_sb[0:W, 0:1])
            nc.tensor.matmul(co_p[0:ocnt, 128:256], convL[0:W, 0:ocnt],
                             w_out_sb[:], start=True, stop=False,
                             skip_group_check=True)
            nc.tensor.matmul(co_p[0:ocnt, 128:256], idb[0:ocnt, 0:ocnt],
                             xc_all[0:ocnt, C * t:C * (t + 1)],
                             start=False, stop=True, skip_group_check=True)
            out_sb = wrk.tile([TS, C], f32, tag="out_sb")
            nc.scalar.copy(out_sb[0:ocnt, :], co_p[0:ocnt, 128:256])
            nc.sync.dma_start(y[TS * t:TS * t + ocnt, :], out_sb[0:ocnt, :])

    nc.compile()
    return nc


def _expected_src_dst():
    i = np.arange(N)
    offs = np.arange(-WIN, WIN + 1)
    j = i[:, None] + offs[None, :]
    valid = ((j // L) == (i[:, None] // L)) & (j >= 0) & (j < N)
    j = np.where(valid, j, i[:, None])
    dst = np.repeat(i, offs.size).astype(np.int32)
    src = j.reshape(-1).astype(np.int32)
    return src, dst


def _host_inputs(x, pos, ori, W_in, Ws0, bs0, Wk, W_out):
    xf = np.ascontiguousarray(x.reshape(N, C), np.float32)
    pos = np.asarray(pos, np.float32)
    ori = np.asarray(ori, np.float32)

    def bf(a):
        return np.asarray(a, BF)

    # Wk rows permuted to (w, c) order, in 6 chunks of 128 rows
    Wk_wc = np.empty_like(Wk)
    for c in range(KC):
        for w in range(W):
            Wk_wc[w * KC + c] = Wk[c * W + w]
    wk_p = np.zeros((128, 6 * W), np.float32)
    for b in range(6):
        wk_p[:, W * b:W * (b + 1)] = Wk_wc[128 * b:128 * (b + 1), :]

    # ws_sb: col-block k holds WS_k rows at partitions 32*(k%3)+d
    ws_sb = np.zeros((128, K18 * KC), np.float32)
    for k in range(K17):
        s = _sidx(k)
        r0 = 32 * (k % 3)
        ws_sb[r0:r0 + 7, KC * k:KC * (k + 1)] = Ws0[s]
        ws_sb[r0 + 7, KC * k:KC * (k + 1)] = bs0[s]
    r0 = 32 * (17 % 3)
    ws_sb[r0 + 3:r0 + 6, KC * 17:KC * 18] = Ws0[5][3:6]
    ws_sb[r0 + 7, KC * 17:KC * 18] = bs0[5]

    # shifted identities S_k[m, j] = 1 iff j = m - ksh
    s_mats = np.zeros((128, K18 * TS), np.float32)
    for k in range(K18):
        ksh = _ksh(k)
        for j in range(TS):
            m = j + ksh
            if 0 <= m < 128:
                s_mats[m, TS * k + j] = 1.0

    common = dict(
        ws_sb=bf(ws_sb), idb=bf(np.eye(128, dtype=np.float32)),
        wk_p=bf(wk_p), s_mats=bf(s_mats), w_out=bf(W_out),
        alph1=np.full((128, 1), NEG_IN, np.float32))

    offs = np.arange(-WIN, WIN + 1)
    in_maps = []
    for ci in range(NCORES):
        s0 = ci * NPC
        g = s0 - WIN + np.arange(HR)
        ok = (g >= 0) & (g < N)
        gi = np.clip(g, 0, N - 1)
        x_pad = np.where(ok[:, None], xf[gi], 0.0).astype(np.float32)
        pos_pad = np.where(ok[:, None], pos[gi], 0.0).astype(np.float32)

        jj, pp = np.meshgrid(np.arange(NT), np.arange(128), indexing="ij")
        rows = (TS * jj + pp)                      # [NT,128] pad-row index
        xl_pad = np.where(x_pad >= 0, x_pad, NEG_IN * x_pad)
        hp = xl_pad @ W_in
        h_pad = np.where(hp >= 0, hp, NEG_IN * hp).astype(np.float32)
        hX_slot = np.repeat(h_pad[rows], KC, axis=2) \
            .transpose(1, 0, 2).reshape(128, NT * W * KC)
        rc = WIN + TS * jj + pp
        okc = rc < HR
        xc_slot = np.where(okc[:, :, None], x_pad[np.minimum(rc, HR - 1)], 0.0)
        xc_slot = xc_slot.transpose(1, 0, 2).reshape(128, NT * C) \
            .astype(np.float32)

        # ---- geometry features on host -> daT layout ----
        gdest = s0 - WIN + rows                    # [NT,128] global dest node
        nb_g = gdest[:, :, None] + offs[None, None, :]
        valid = ((nb_g // L) == (gdest[:, :, None] // L)) \
            & (nb_g >= 0) & (nb_g < N)
        nb_gi = np.where(valid, np.clip(nb_g, 0, N - 1),
                         np.clip(gdest[:, :, None], 0, N - 1))
        pos_c = pos_pad[rows]                              # [NT,128,3]
        ori_c = ori[np.clip(gdest, 0, N - 1)] \
            .reshape(NT, 128, 3, 3)
        ori_c = np.where((gdest >= 0)[:, :, None, None]
                         & (gdest < N)[:, :, None, None], ori_c, 0.0)
        nbp_a = np.where(valid[..., None], pos[nb_gi], pos_c[:, :, None, :])
        nbo_a = np.where(valid[..., None], ori[nb_gi], 0.0) \
            .reshape(NT, 128, K17, 3, 3)
        mask = valid.astype(np.float32)
        ncl = (K17 - mask.sum(-1)).astype(np.float32)       # [NT,128]

        D = nbp_a - pos_c[:, :, None, :]                    # [NT,128,17,3]
        d2 = (D * D).sum(-1)
        rs = 1.0 / (np.sqrt(d2) + 1e-9)
        distR = np.sqrt(d2) / R
        dirn = D * rs[..., None]
        local = np.einsum('tpab,tpkb->tpka', ori_c, dirn)
        ofeat = np.einsum('tpab,tpkab->tpka', ori_c, nbo_a)
        da = np.zeros((NT, 128, K18, 8), np.float32)
        da[:, :, :K17, 0:3] = local
        da[:, :, :K17, 3:6] = ofeat
        da[:, :, :K17, 6] = distR
        da[:, :, :K17, 7] = mask
        da[:, :, 17, 3:6] = ncl[..., None] * ofeat[:, :, 8, :]
        da[:, :, 17, 7] = ncl

        # daT layout: row 32*(k%3)+d, col 144*(k//3) + 8 + slot-node
        daT = np.zeros((NT, 128, DTC), np.float32)
        for k in range(K18):
            r0, g6 = 32 * (k % 3), 144 * (k // 3)
            daT[:, r0:r0 + 8, g6 + 8:g6 + 136] = \
                da[:, :, k, :].transpose(0, 2, 1)
        daT_slot = daT.transpose(1, 0, 2).reshape(128, NT * DTC)

        in_maps.append(dict(
            hX_slot=bf(hX_slot),
            xc_slot=bf(xc_slot),
            daT_slot=bf(daT_slot),
            **common))
    return in_maps


def kernel(x, pos, seq, ori, W_in, Ws0, bs0, Wk, W_out, src, dst):
    exp_src, exp_dst = _expected_src_dst()
    assert np.array_equal(np.asarray(src), exp_src), "unexpected src graph"
    assert np.array_equal(np.asarray(dst), exp_dst), "unexpected dst graph"

    from concourse.bass_utils import run_bass_kernel_spmd

    if "nc" not in _PROG:
        _PROG["nc"] = _build_program()
    nc = _PROG["nc"]

    in_maps = _host_inputs(np.asarray(x), np.asarray(pos), np.asarray(ori),
                           np.asarray(W_in), np.asarray(Ws0), np.asarray(bs0),
                           np.asarray(Wk), np.asarray(W_out))
    res = run_bass_kernel_spmd(nc, in_maps, list(range(NCORES)))
    out = np.concatenate([res.results[i]["y"] for i in range(NCORES)], axis=0)
    return out.reshape(B, L, C).astype(np.float32)
